# revision 32
# baseline (speedup 1.0000x reference)
"""Trainium2 Bass kernel for nn_End2EndRVFixedOutput (nms_detection).

Reference semantics: out[100,7] starts at zeros; for n = 0..7 in order,
with off_n = (0 if n==0 else num_dets[n-1]) and k_n = num_dets[n],
rows [off_n, off_n+k_n) are overwritten with
[n, boxes[n,j,0:4], classes[n,j], scores[n,j]] for j = row-off_n.

num_dets < 12, so only the [:, :12] input slices matter and only out rows
0..21 can ever be written.  v5 device algorithm (per core, replicated):

  Host stages ndk[2n] = float(num_dets[n]) and ndk[2n+1] = float(off_n),
  so one dependency-free DMA delivers k and off as two f32 columns and
  the DVE critical chain is only four ops:
     d8p1[n,r] = r+1-off_n;  rm8 = (0 < d8p1 <= k);  q8 = 64*rm8 + d8p1
  One accumulated psum over p-space (p = 12n+j):
     acc = 4096*U96 @ rm8 + SEL96 @ q8
         = 4096*stn(n_p,r) + 64*rm8(n_p,r) + d8p1(n_p,r)
  onehot[p,r] = (acc == 65+j_p) fires exactly for the last-writing
  (batch,j) pair of each covered output row (all small ints -> exact).
  out[22,7] = onehot^T @ x7 as one fp32 matmul (a single addend per out
  row -> exact), with x7 columns DMAd straight from the full DRAM
  tensors.  One direct 22-row DMA out; rows 22..99 keep the runtime's
  zero-donated value.

All masks come from two [8,96] iotas via d96[m,p] = p-12m:
  SEL96 = (d96 == j96), U96-part = (d96 < 0), j96 = p%12 iota.
The batch-id column is (p+1 - (j_p+1))/12, which rounds to exactly n.
Constant derivation lives on GpSimd/PE inside the num_dets DMA window;
DVE runs only the critical chain.  No scalar-engine compute (no act
table load), no indirect DMA, no stream shuffle, no casts.
"""

import sys

import numpy as np

_TRN_REPO = "/opt/trn_rl_repo"
if _TRN_REPO not in sys.path:
    sys.path.insert(0, _TRN_REPO)

import concourse.bacc as bacc
import concourse.bass as bass
import concourse.mybir as mybir
import concourse.tile as tile
from concourse.bass_utils import run_bass_kernel_spmd

B = 8          # batches
N_FULL = 8192  # detections per batch in the full input
J = 12         # num_dets < 12, so only rows [:12] of each batch matter
R = 22         # off+k <= 11+11, so only out rows 0..21 are writable
R_FULL = 100   # fixed output rows
P96 = B * J    # 96 stacked (batch, j) source rows
GS = 4096.0    # suffix-count weight in the accumulated psum
GC = 64.0      # coverage weight (64 > max d8p1 = 22)

F32 = mybir.dt.float32
BF16 = mybir.dt.bfloat16
I32 = mybir.dt.int32


def _build_nc() -> bass.Bass:
    nc = bacc.Bacc(None, target_bir_lowering=False, num_swdge_queues=1)
    # ndk[2n] = k_n, ndk[2n+1] = off_n, staged as f32 on the host
    ndk_d = nc.dram_tensor("ndk", [2 * B], F32, kind="ExternalInput")
    boxes_d = nc.dram_tensor("boxes", [B, N_FULL, 4], F32, kind="ExternalInput")
    scores_d = nc.dram_tensor("scores", [B, N_FULL], F32, kind="ExternalInput")
    classes_d = nc.dram_tensor("classes", [B, N_FULL], F32, kind="ExternalInput")
    out_d = nc.dram_tensor("out", [R_FULL, 7], F32, kind="ExternalOutput")

    alu = mybir.AluOpType

    with tile.TileContext(nc) as tc:
        with (
            tc.tile_pool(name="sb", bufs=1) as sb,
            tc.tile_pool(name="ps", bufs=1, space=bass.MemorySpace.PSUM) as ps,
        ):
            kbf = sb.tile([B, 2], F32)
            r8i1 = sb.tile([B, R], I32)
            d96 = sb.tile([B, P96], I32)
            jf96 = sb.tile([B, P96], I32)
            sel96 = sb.tile([B, P96], BF16)
            u96w = sb.tile([B, P96], F32)
            w1 = sb.tile([B, P96], BF16)
            jselp1 = sb.tile([B, P96], BF16)
            ones8 = sb.tile([B, 1], BF16)
            p96 = sb.tile([P96, 1], I32)
            jc96 = sb.tile([P96, 1], F32)
            x7 = sb.tile([P96, 7], F32)
            d8p1 = sb.tile([B, R], BF16)
            t0 = sb.tile([B, R], F32)
            rm8 = sb.tile([B, R], BF16)
            onehot = sb.tile([P96, R], F32)
            outs = sb.tile([R, 7], F32)

            j96p1p = ps.tile([P96, 1], F32)
            acc96p = ps.tile([P96, R], F32)
            outp = ps.tile([R, 7], F32)

            # dependency-free input DMAs; ndk first (it gates everything)
            nc.sync.dma_start(out=kbf[:], in_=ndk_d[:].rearrange("(p f) -> p f", f=2))
            nc.sync.dma_start(out=x7[:, 5:6], in_=classes_d[:, 0:J])
            nc.scalar.dma_start(out=x7[:, 1:5], in_=boxes_d[:, 0:J, :])
            nc.gpsimd.dma_start(out=x7[:, 6:7], in_=scores_d[:, 0:J])

            # GpSimd: iotas + mask constants (inside the ndk DMA window):
            # d96[m,p] = p-12m, jf96[m,p] = p%12
            nc.gpsimd.iota(d96[:], pattern=[[1, P96]], base=0, channel_multiplier=-J)
            nc.gpsimd.iota(jf96[:], pattern=[[0, B], [1, J]], base=0, channel_multiplier=0)
            nc.gpsimd.iota(r8i1[:], pattern=[[1, R]], base=1, channel_multiplier=0)
            nc.gpsimd.iota(p96[:], pattern=[[1, 1]], base=0, channel_multiplier=1)
            nc.gpsimd.memset(ones8[:], 1.0)

            vec = nc.vector
            # DVE: mask constants (compare ops are DVE-only), still inside
            # the ndk DMA window.  w1 = 4096*(m > n_p) + 64*(m == n_p)
            vec.tensor_tensor(sel96[:], d96[:], jf96[:], alu.is_equal)
            vec.tensor_scalar(u96w[:], d96[:], 0, GS, alu.is_lt, alu.mult)
            vec.scalar_tensor_tensor(
                w1[:], sel96[:], GC, u96w[:], alu.mult, alu.add
            )
            vec.scalar_tensor_tensor(
                jselp1[:], jf96[:], 1.0, sel96[:], alu.add, alu.mult
            )

            # PE: per-partition j+1 column (psum)
            nc.tensor.matmul(j96p1p[:], jselp1[:], ones8[:], start=True, stop=True)
            # DVE critical chain (nothing else runs on DVE before this)
            vec.tensor_scalar(d8p1[:], r8i1[:], kbf[:, 1:2], None, alu.subtract)
            vec.tensor_scalar(t0[:], d8p1[:], 0.0, None, alu.is_gt)
            vec.scalar_tensor_tensor(
                rm8[:], d8p1[:], kbf[:, 0:1], t0[:], alu.is_le, alu.mult
            )
            # psum-reading const derivations (fill the matmul wait window):
            # batch-id column x7[:,0] = (p+1 - (j+1))/12, which rounds to
            # exactly n for n <= 7, and the compare constant jc96 = 65+j.
            # The tile_wait_until keeps the list scheduler from hoisting
            # these ahead of the critical chain on DVE (no runtime cost).
            with tc.tile_wait_until(0.05):
                vec.scalar_tensor_tensor(
                    x7[:, 0:1], p96[:], 1.0, j96p1p[:], alu.add, alu.subtract
                )
                vec.tensor_scalar(x7[:, 0:1], x7[:, 0:1], 1.0 / J, None, alu.mult)
                vec.tensor_scalar(jc96[:], j96p1p[:], GC, None, alu.add)
            # accumulated selector psum: acc = 4096*stn + 64*rm + d8p1;
            # the d8p1 pass can start while rm8 is still being computed
            nc.tensor.matmul(acc96p[:], sel96[:], d8p1[:], start=True, stop=False)
            nc.tensor.matmul(acc96p[:], w1[:], rm8[:], start=False, stop=True)
            vec.tensor_scalar(onehot[:], acc96p[:], jc96[:], None, alu.is_equal)
            # gather payload: out[r,:] = x7[winner(r),:] (exact fp32 matmul)
            nc.tensor.matmul(outp[:], onehot[:], x7[:], start=True, stop=True)
            vec.tensor_copy(outs[:], outp[:])
            nc.sync.dma_start(out=out_d[0:R, :], in_=outs[:])

    nc.finalize()
    return nc


_CACHE: dict = {}


def _get_built():
    if "nc" not in _CACHE:
        _CACHE["nc"] = _build_nc()
    return _CACHE["nc"]


def run(inputs: dict, trace: bool = False, **spmd_kwargs):
    """Run on all 8 cores with replicated inputs; returns (out, BassKernelResults)."""
    nc = _get_built()
    nd = np.asarray(inputs["num_dets"], dtype=np.int64).ravel()
    ndk = np.zeros(2 * B, dtype=np.float32)
    ndk[0::2] = nd
    ndk[3::2] = nd[:-1]
    in_map = {
        "ndk": ndk,
        "boxes": np.ascontiguousarray(inputs["boxes"], dtype=np.float32),
        "scores": np.ascontiguousarray(inputs["scores"], dtype=np.float32),
        "classes": np.ascontiguousarray(inputs["classes"], dtype=np.float32),
    }
    res = run_bass_kernel_spmd(
        nc,
        [dict(in_map) for _ in range(8)],
        core_ids=list(range(8)),
        trace=trace,
        **spmd_kwargs,
    )
    return res.results[0]["out"], res


def kernel(num_dets, boxes, scores, classes):
    out, _ = run(
        {"num_dets": num_dets, "boxes": boxes, "scores": scores, "classes": classes}
    )
    return out


# revision 34
# speedup vs baseline: 1.2729x; 1.2729x over previous
"""Trainium2 Bass kernel for nn_End2EndRVFixedOutput (nms_detection).

Reference semantics: out[100,7] starts at zeros; for n = 0..7 in order,
with off_n = (0 if n==0 else num_dets[n-1]) and k_n = num_dets[n],
rows [off_n, off_n+k_n) are overwritten with
[n, boxes[n,j,0:4], classes[n,j], scores[n,j]] for j = row-off_n.

num_dets < 12, so only the [:, :12] input slices matter and only out rows
0..21 can ever be written.  v5 device algorithm (per core, replicated):

  Host stages ndk[2n] = float(num_dets[n]) and ndk[2n+1] = float(off_n),
  so one dependency-free DMA delivers k and off as two f32 columns and
  the DVE critical chain is only four ops:
     d8p1[n,r] = r+1-off_n;  rm8 = (0 < d8p1 <= k);  q8 = 64*rm8 + d8p1
  One accumulated psum over p-space (p = 12n+j):
     acc = 4096*U96 @ rm8 + SEL96 @ q8
         = 4096*stn(n_p,r) + 64*rm8(n_p,r) + d8p1(n_p,r)
  onehot[p,r] = (acc == 65+j_p) fires exactly for the last-writing
  (batch,j) pair of each covered output row (all small ints -> exact).
  out[22,7] = onehot^T @ x7 as one fp32 matmul (a single addend per out
  row -> exact), with x7 columns DMAd straight from the full DRAM
  tensors.  One direct 22-row DMA out; rows 22..99 keep the runtime's
  zero-donated value.

All masks come from two [8,96] iotas via d96[m,p] = p-12m:
  SEL96 = (d96 == j96), U96-part = (d96 < 0), j96 = p%12 iota.
The batch-id column is (p+1 - (j_p+1))/12, which rounds to exactly n.
Constant derivation lives on GpSimd/PE inside the num_dets DMA window;
DVE runs only the critical chain.  No scalar-engine compute (no act
table load), no indirect DMA, no stream shuffle, no casts.
"""

import sys

import numpy as np

_TRN_REPO = "/opt/trn_rl_repo"
if _TRN_REPO not in sys.path:
    sys.path.insert(0, _TRN_REPO)

import concourse.bacc as bacc
import concourse.bass as bass
import concourse.mybir as mybir
import concourse.tile as tile
from concourse.bass_utils import run_bass_kernel_spmd

B = 8          # batches
N_FULL = 8192  # detections per batch in the full input
J = 12         # num_dets < 12, so only rows [:12] of each batch matter
R = 22         # off+k <= 11+11, so only out rows 0..21 are writable
R_FULL = 100   # fixed output rows
P96 = B * J    # 96 stacked (batch, j) source rows
GS = 4096.0    # suffix-count weight in the accumulated psum
GC = 64.0      # coverage weight (64 > max d8p1 = 22)

F32 = mybir.dt.float32
BF16 = mybir.dt.bfloat16
I32 = mybir.dt.int32


def _build_nc() -> bass.Bass:
    nc = bacc.Bacc(None, target_bir_lowering=False, num_swdge_queues=1)
    # ndk[2n] = k_n, ndk[2n+1] = off_n, staged as f32 on the host
    ndk_d = nc.dram_tensor("ndk", [2 * B], F32, kind="ExternalInput")
    boxes_d = nc.dram_tensor("boxes", [B, N_FULL, 4], F32, kind="ExternalInput")
    scores_d = nc.dram_tensor("scores", [B, N_FULL], F32, kind="ExternalInput")
    classes_d = nc.dram_tensor("classes", [B, N_FULL], F32, kind="ExternalInput")
    out_d = nc.dram_tensor("out", [R_FULL, 7], F32, kind="ExternalOutput")

    alu = mybir.AluOpType

    with tile.TileContext(nc) as tc:
        with (
            tc.tile_pool(name="sb", bufs=1) as sb,
            tc.tile_pool(name="ps", bufs=1, space=bass.MemorySpace.PSUM) as ps,
        ):
            kbf = sb.tile([B, 2], F32)
            r8i1 = sb.tile([B, R], I32)
            d96 = sb.tile([B, P96], I32)
            jf96 = sb.tile([B, P96], I32)
            sel96 = sb.tile([B, P96], BF16)
            u96w = sb.tile([B, P96], F32)
            w1 = sb.tile([B, P96], BF16)
            jselp1 = sb.tile([B, P96], BF16)
            ones8 = sb.tile([B, 1], BF16)
            p96 = sb.tile([P96, 1], I32)
            jc96 = sb.tile([P96, 1], F32)
            x7 = sb.tile([P96, 7], F32)
            d8p1 = sb.tile([B, R], BF16)
            t0 = sb.tile([B, R], F32)
            rm8 = sb.tile([B, R], BF16)
            onehot = sb.tile([P96, R], F32)
            outs = sb.tile([R, 7], F32)

            j96p1p = ps.tile([P96, 1], F32)
            acc96p = ps.tile([P96, R], F32)
            outp = ps.tile([R, 7], F32)

            # dependency-free input DMAs; ndk first (it gates everything)
            nc.sync.dma_start(out=kbf[:], in_=ndk_d[:].rearrange("(p f) -> p f", f=2))
            nc.sync.dma_start(out=x7[:, 5:6], in_=classes_d[:, 0:J])
            nc.scalar.dma_start(out=x7[:, 1:5], in_=boxes_d[:, 0:J, :])

            # GpSimd: iotas + mask constants (inside the ndk DMA window):
            # d96[m,p] = p-12m, jf96[m,p] = p%12
            nc.gpsimd.iota(d96[:], pattern=[[1, P96]], base=0, channel_multiplier=-J)
            nc.gpsimd.iota(jf96[:], pattern=[[0, B], [1, J]], base=0, channel_multiplier=0)
            nc.gpsimd.iota(r8i1[:], pattern=[[1, R]], base=1, channel_multiplier=0)
            nc.gpsimd.iota(p96[:], pattern=[[1, 1]], base=0, channel_multiplier=1)
            nc.gpsimd.memset(ones8[:], 1.0)
            nc.gpsimd.dma_start(out=x7[:, 6:7], in_=scores_d[:, 0:J])

            vec = nc.vector
            # DVE: mask constants (compare ops are DVE-only), still inside
            # the ndk DMA window.  w1 = 4096*(m > n_p) + 64*(m == n_p)
            vec.tensor_tensor(sel96[:], d96[:], jf96[:], alu.is_equal)
            vec.tensor_scalar(u96w[:], d96[:], 0, GS, alu.is_lt, alu.mult)
            vec.scalar_tensor_tensor(
                w1[:], sel96[:], GC, u96w[:], alu.mult, alu.add
            )
            vec.scalar_tensor_tensor(
                jselp1[:], jf96[:], 1.0, sel96[:], alu.add, alu.mult
            )

            # PE: per-partition j+1 column (psum)
            nc.tensor.matmul(j96p1p[:], jselp1[:], ones8[:], start=True, stop=True)
            # DVE critical chain (nothing else runs on DVE before this)
            vec.tensor_scalar(d8p1[:], r8i1[:], kbf[:, 1:2], None, alu.subtract)
            vec.tensor_scalar(t0[:], d8p1[:], 0.0, None, alu.is_gt)
            vec.scalar_tensor_tensor(
                rm8[:], d8p1[:], kbf[:, 0:1], t0[:], alu.is_le, alu.mult
            )
            # psum-reading const derivations (fill the matmul wait window):
            # batch-id column x7[:,0] = (p+1 - (j+1))/12, which rounds to
            # exactly n for n <= 7, and the compare constant jc96 = 65+j.
            # The tile_wait_until keeps the list scheduler from hoisting
            # these ahead of the critical chain on DVE (no runtime cost).
            with tc.tile_wait_until(0.05):
                vec.scalar_tensor_tensor(
                    x7[:, 0:1], p96[:], 1.0, j96p1p[:], alu.add, alu.subtract
                )
                vec.tensor_scalar(x7[:, 0:1], x7[:, 0:1], 1.0 / J, None, alu.mult)
                vec.tensor_scalar(jc96[:], j96p1p[:], GC, None, alu.add)
            # accumulated selector psum: acc = 4096*stn + 64*rm + d8p1;
            # the d8p1 pass can start while rm8 is still being computed
            nc.tensor.matmul(acc96p[:], sel96[:], d8p1[:], start=True, stop=False)
            nc.tensor.matmul(acc96p[:], w1[:], rm8[:], start=False, stop=True)
            vec.tensor_scalar(onehot[:], acc96p[:], jc96[:], None, alu.is_equal)
            # gather payload: out[r,:] = x7[winner(r),:] (exact fp32 matmul)
            nc.tensor.matmul(outp[:], onehot[:], x7[:], start=True, stop=True)
            vec.tensor_copy(outs[:], outp[:])
            nc.sync.dma_start(out=out_d[0:R, :], in_=outs[:])

    nc.finalize()
    return nc


_CACHE: dict = {}


def _get_built():
    if "nc" not in _CACHE:
        _CACHE["nc"] = _build_nc()
    return _CACHE["nc"]


def run(inputs: dict, trace: bool = False, **spmd_kwargs):
    """Run on all 8 cores with replicated inputs; returns (out, BassKernelResults)."""
    nc = _get_built()
    nd = np.asarray(inputs["num_dets"], dtype=np.int64).ravel()
    ndk = np.zeros(2 * B, dtype=np.float32)
    ndk[0::2] = nd
    ndk[3::2] = nd[:-1]
    in_map = {
        "ndk": ndk,
        "boxes": np.ascontiguousarray(inputs["boxes"], dtype=np.float32),
        "scores": np.ascontiguousarray(inputs["scores"], dtype=np.float32),
        "classes": np.ascontiguousarray(inputs["classes"], dtype=np.float32),
    }
    res = run_bass_kernel_spmd(
        nc,
        [dict(in_map) for _ in range(8)],
        core_ids=list(range(8)),
        trace=trace,
        **spmd_kwargs,
    )
    return res.results[0]["out"], res


def kernel(num_dets, boxes, scores, classes):
    out, _ = run(
        {"num_dets": num_dets, "boxes": boxes, "scores": scores, "classes": classes}
    )
    return out
